# revision 8
# baseline (speedup 1.0000x reference)
"""ViT attention block (B=8, N=1024, dim=1024, heads=16, d_k=64) on 8 trn2 NeuronCores.

Sharding: data-parallel over batch (1 batch per core), weights replicated.
No collectives needed; each core computes its batch's full attention output.

Per-core algorithm (all matmuls on TensorE contract over the partition dim):
  - host pre-transposes x[b] -> xT [dim, tokens] so QKV projections can use
    w_qkv (natural layout) as the stationary operand.
  - QT/KT = (w_qkv[:, :2048]).T @ xT  -> [2048, tokens]; head pair 2t,2t+1
    lives in partition-tile t ([128, 1024]), i.e. heads' d_k=64 rows stacked.
  - V = xT.T @ w_qkv[:, 2048:]       -> [tokens, 1024], stored with a
    constant-1 column appended per head (65 cols/head) so the PV matmul
    produces softmax row-sums for free.
  - per head pair: S^T[m,n] = (KT tile).T @ QT (K=64 contraction; the two
    heads run as concurrent row-group matmuls via tile_position). exp via
    ScalarE streams per [128,1024] chunk into ET bf16. ScalarE exp is the
    per-pair pacing engine (16 x 1.15us = 18.4us/pair), so every pair loop
    is packed with >= that much TensorE work (V-nh1 chunks ride loops 1-4,
    out-projection wave A rides loops 5-7).
  - PV: out^T[d'+1, n] = V'.T @ E^T accumulated over m tiles; row 64 is the
    softmax denominator. Stage to SBUF, reciprocal on a [128, 8] reshape,
    DRAM-bounce broadcast back, normalize multiplies on GpSimd/DVE.
  - final = attnT.T @ w_out + b_out in three waves: wave A (pairs 0-3,
    bias folded) -> oacc during loops 5-7; wave B (pairs 4-6) -> oacc2 and
    wave C (pair 7) -> out in the drain, pipelined so the out DMAs overlap
    the wave MMs. Output rides DRAM as bf16 (halves the 4MB tail DMA).

Startup: ~28 garbage warm-up matmuls issue immediately after the framework
preamble so the PE HAM clock-gate reaches 2.4 GHz before the first real
matmul. Input DMAs are priority-ordered: xT (sync), wq0 (scalar, 2 desc),
wk0 (vector, 2 desc), wv nh0-half then nh1-half (gpsimd); wout/bias issue
at loop 2 so they don't steal startup HBM bandwidth.
"""

import os
import numpy as np
import ml_dtypes

import concourse.bass as bass
from concourse import bacc
import concourse.mybir as mybir
import concourse.tile as tile
from concourse.bass_utils import run_bass_kernel_spmd

P = 128
N_TOK = 1024
DIM = 1024
HEADS = 16
D_K = 64
N_CORES = 8
SCALE = D_K ** -0.5  # 0.125

NP_T = N_TOK // P   # 8 token tiles
DP = DIM // P       # 8 dim tiles
NPAIRS = HEADS // 2  # 8 head pairs
VW = D_K + 1        # 65: V columns per head incl. ones column

# matmul operand dtype: "bf16" | "fp32r" | "fp32"
MM_DTYPE = os.environ.get("KERNEL_MM_DTYPE", "bf16")
_DT = {
    "bf16": mybir.dt.bfloat16,
    "fp32r": mybir.dt.float32r,
    "fp32": mybir.dt.float32,
}[MM_DTYPE]
_NPDT = {"bf16": ml_dtypes.bfloat16, "fp32r": np.float32, "fp32": np.float32}[MM_DTYPE]

F32 = mybir.dt.float32

N_WARMUP = 28  # garbage matmuls to flip the HAM clock gate early


def build_program():
    nc = bacc.Bacc("TRN2", target_bir_lowering=False, debug=False)

    xT = nc.dram_tensor("xT", [DIM, N_TOK], _DT, kind="ExternalInput").ap()
    wqkv = nc.dram_tensor("w_qkv", [DIM, 3 * DIM], _DT, kind="ExternalInput").ap()
    wout = nc.dram_tensor("w_out", [DIM, DIM], _DT, kind="ExternalInput").ap()
    bout = nc.dram_tensor("b_out", [DIM], F32, kind="ExternalInput").ap()
    out = nc.dram_tensor("out", [N_TOK, DIM], _DT, kind="ExternalOutput").ap()
    # reciprocal'd softmax denominator bounce rows (one per head)
    rs_dram = nc.dram_tensor("rs_scratch", [HEADS, N_TOK], F32).ap()

    def wq_block(j):
        """One DMA-able view of w_qkv column block j (128 cols) across all
        1024 rows: [p=128, k=8, c=128] -> lands as an SBUF [128, 1024] tile
        whose k-th 128-col slice is w_qkv[k*128:(k+1)*128, j*128:(j+1)*128]."""
        return bass.AP(
            tensor=wqkv.tensor, offset=wqkv.offset + j * P,
            ap=[[3 * DIM, P], [P * 3 * DIM, DP], [1, P]],
        )

    def wq_halfblock(j, khalf):
        """Half of wq_block: k-tiles khalf*4..khalf*4+3 of column block j."""
        return bass.AP(
            tensor=wqkv.tensor,
            offset=wqkv.offset + j * P + khalf * 4 * P * 3 * DIM,
            ap=[[3 * DIM, P], [P * 3 * DIM, 4], [1, P]],
        )

    def wv_halfsrc(k0, nh):
        """V-weight nh-half for k-tiles k0, k0+1: [128, 2*512]."""
        return bass.AP(
            tensor=wqkv.tensor,
            offset=wqkv.offset + k0 * P * 3 * DIM + 2 * DIM + nh * 512,
            ap=[[3 * DIM, P], [P * 3 * DIM, 2], [1, 512]])

    with tile.TileContext(nc) as tc:
        with (
            tc.tile_pool(name="persist", bufs=1) as persist,
            tc.tile_pool(name="qkt", bufs=4) as qktp,
            tc.tile_pool(name="wqkb", bufs=2) as wqkb,
            tc.tile_pool(name="wvoa", bufs=1) as wvoa,
            tc.tile_pool(name="et", bufs=16) as etp,
            tc.tile_pool(name="stg", bufs=3) as stgp,
            tc.tile_pool(name="small", bufs=2) as small,
            tc.tile_pool(name="oap", bufs=7) as oap,
        ):
            v_sb = []      # per token-tile: [128, 16*65]
            attnT_sb = []  # per pair: [128, 1024] = two heads' [64, n]
            for j in range(NP_T):
                v_sb.append(persist.tile([P, HEADS * VW], _DT, tag=f"v{j}",
                                         name=f"v{j}"))
            for p in range(NPAIRS):
                attnT_sb.append(persist.tile([P, N_TOK], _DT, tag=f"attnT{p}",
                                             name=f"attnT{p}"))

            # ---- input DMAs, priority ordered across 4 queues ----
            # sync: xT (the phase-1 pacing stream); scalar: wq0 halves;
            # vector: wk0 halves; gpsimd: wv nh0-half then nh1-half.
            # wout/bias issue later (loop 2) so they don't steal startup BW.
            xT_all = persist.tile([P, DP * N_TOK], _DT, tag="xT", name="xT")
            for k in range(DP):
                nc.sync.dma_start(
                    xT_all[:, k * N_TOK:(k + 1) * N_TOK],
                    xT[k * P:(k + 1) * P, :])

            def xs(k):
                return xT_all[:, k * N_TOK:(k + 1) * N_TOK]

            wq_cur = wqkb.tile([P, DIM], _DT, tag="wq", name="wq0")
            wk_cur = wqkb.tile([P, DIM], _DT, tag="wk", name="wk0")
            for kh in range(2):
                nc.scalar.dma_start(wq_cur[:, kh * 512:(kh + 1) * 512],
                                    wq_halfblock(0, kh))
                nc.gpsimd.dma_start(wk_cur[:, kh * 512:(kh + 1) * 512],
                                    wq_halfblock(DP, kh))

            # wv layout: [128, 2*4096]: nh-half major, then k-major 512-col
            # blocks: wv_all[:, nh*4096 + k*512 : +512] = w_qkv V columns
            # [k*128:(k+1)*128, 2048+nh*512 : 2048+(nh+1)*512]
            wv_all = wvoa.tile([P, DP * DIM], _DT, tag="wvoa", name="wv")
            for nh in range(2):
                for k0 in range(0, DP, 2):
                    nc.gpsimd.dma_start(
                        wv_all[:, nh * 4096 + k0 * 512: nh * 4096 + (k0 + 2) * 512],
                        wv_halfsrc(k0, nh))

            def wvs(k, nh):
                return wv_all[:, nh * 4096 + k * 512:nh * 4096 + (k + 1) * 512]

            wout_all = persist.tile([P, DP * DIM], _DT, tag="wout",
                                    name="wout")
            bias_bc = persist.tile([P, DIM], F32, tag="bias")

            def issue_wout_bias():
                for k0 in range(0, DP, 2):
                    src = bass.AP(
                        tensor=wout.tensor, offset=wout.offset + k0 * P * DIM,
                        ap=[[DIM, P], [P * DIM, 2], [1, DIM]])
                    nc.gpsimd.dma_start(
                        wout_all[:, k0 * DIM:(k0 + 2) * DIM], src)
                bias_in = bass.AP(tensor=bout.tensor, offset=bout.offset,
                                  ap=[[0, P]] + list(bout.ap))
                nc.gpsimd.dma_start(bias_bc[:], bias_in)

            def wouts(k):
                return wout_all[:, k * DIM:(k + 1) * DIM]

            for j in range(NP_T):
                nc.vector.memset(
                    v_sb[j][:].rearrange("p (h x) -> p h x", x=VW)[:, :, D_K:],
                    1.0)

            # ============ phase 0: HAM warm-up. Garbage matmuls on a
            # memset tile into a scratch psum bank keep the PE busy from
            # the end of the framework preamble so the clock gate flips
            # to 8/8 before the first real matmul. ============
            with tc.tile_pool(name="pwarm", bufs=1, space="PSUM") as pwarm, \
                    tc.tile_pool(name="pq1", bufs=2, space="PSUM") as pq1:
                wtile = small.tile([P, P], _DT, tag="warm", name="warmsrc")
                nc.vector.memset(wtile[:], 0.0)
                wps = pwarm.tile([P, P], F32, tag="warm", name="warmps")
                for _ in range(N_WARMUP):
                    nc.tensor.matmul(wps[:], lhsT=wtile[:], rhs=wtile[:],
                                     start=True, stop=True)

                # ============ phase 1: pair-0 QT/KT, k-interleaved so both
                # psums accumulate as the xT / weight DMAs arrive ==========
                psq = pq1.tile([P, N_TOK], F32, tag="pq", name="psqk0")
                psk = pq1.tile([P, N_TOK], F32, tag="pq", name="psqk8")
                for k in range(DP):
                    for ps, wblk in ((psq, wq_cur), (psk, wk_cur)):
                        for nh in range(2):
                            nc.tensor.matmul(
                                ps[:, nh * 512:(nh + 1) * 512],
                                lhsT=wblk[:, k * P:(k + 1) * P],
                                rhs=xs(k)[:, nh * 512:(nh + 1) * 512],
                                start=(k == 0), stop=(k == DP - 1),
                            )
                qt_cur = qktp.tile([P, N_TOK], _DT, tag="qkt", name="qkt0")
                nc.vector.tensor_copy(out=qt_cur[:], in_=psq[:])
                kt_cur = qktp.tile([P, N_TOK], _DT, tag="qkt", name="qkt8")
                nc.vector.tensor_copy(out=kt_cur[:], in_=psk[:])

            # ============ phase 2: pipelined attention ============
            with (
                tc.tile_pool(name="pst", bufs=2, space="PSUM") as pstp,
                tc.tile_pool(name="pfill", bufs=1, space="PSUM") as pfill,
                tc.tile_pool(name="pwa", bufs=1, space="PSUM") as pwa,
                tc.tile_pool(name="pgen", bufs=2, space="PSUM") as pgen,
            ):
                et_tiles = {}   # (pair, mt) -> ET tile [128, 2048]
                inflight = {}   # accumulation state for pv / filler psums

                def emit_v_chunk(j, nh):
                    """8 matmuls: V[j-tile, nh-half] = xT.T @ wv, evicted
                    into v_sb[j] (heads nh*8..nh*8+7, 64 cols each)."""
                    ps = pgen.tile([P, 512], F32, tag="pv",
                                   name=f"psv{j}_{nh}")
                    for k in range(DP):
                        nc.tensor.matmul(
                            ps[:],
                            lhsT=xs(k)[:, j * P:(j + 1) * P],
                            rhs=wvs(k, nh),
                            start=(k == 0), stop=(k == DP - 1),
                        )
                    nc.vector.tensor_copy(
                        out=v_sb[j][:].rearrange(
                            "p (h x) -> p h x", x=VW)[:, 8 * nh:8 * nh + 8,
                                                      :D_K],
                        in_=ps[:].rearrange("p (h d) -> p h d", d=D_K),
                    )

                def normalize_evict(p, h, stg):
                    """Normalize the staged PV result by the softmax
                    denominator (row 64) and write into attnT_sb[p].
                    Latency chains ride the gpsimd SWDGE queue; the last
                    pair uses the (by then idle) sync queue."""
                    hg = 2 * p + h
                    dma = nc.gpsimd.dma_start if p < NPAIRS - 1 else \
                        nc.sync.dma_start
                    rsp = small.tile([P, NP_T], F32, tag="rsp",
                                     name=f"rsp{hg}")
                    dma(rsp[:], stg[D_K:VW, :].rearrange(
                        "o (p i) -> o p i", p=P))
                    rsq = small.tile([P, NP_T], F32, tag="rsq",
                                     name=f"rsq{hg}")
                    nc.vector.reciprocal(rsq[:], rsp[:])
                    dma(rs_dram[hg].rearrange("(p i) -> p i", p=P), rsq[:])
                    rs_row = rs_dram[hg:hg + 1, :]
                    rs_bc = bass.AP(tensor=rs_row.tensor, offset=rs_row.offset,
                                    ap=[[0, D_K], list(rs_row.ap)[-1]])
                    rrec = small.tile([D_K, N_TOK], F32, tag="rrec",
                                      name=f"rrec{hg}")
                    dma(rrec[:], rs_bc)
                    if h == 0:
                        nc.vector.tensor_mul(out=attnT_sb[p][0:D_K, :],
                                             in0=stg[0:D_K, :], in1=rrec[:])
                    else:
                        tmp = small.tile([D_K, N_TOK], _DT, tag="oddtmp",
                                         name=f"oddtmp{hg}")
                        nc.vector.tensor_mul(out=tmp[:],
                                             in0=stg[0:D_K, :], in1=rrec[:])
                        dma(attnT_sb[p][D_K:P, :], tmp[:])

                def pv_group(p, h, nh, mts, start, stop, use_scalar_evict):
                    """PV matmuls for (pair, head, nh) over the given mt
                    list, accumulated in a pgen psum. On stop: evict into
                    the stg half; on nh==1 stop: launch the normalize."""
                    hg = 2 * p + h
                    key = (p, h, nh)
                    if start:
                        inflight[key] = pgen.tile(
                            [P, 512], F32, tag="pv", name=f"pv{p}_{h}_{nh}")
                    pvt = inflight[key]
                    for mt in mts:
                        et = et_tiles[(p, mt)]
                        nc.tensor.matmul(
                            pvt[0:VW, :],
                            lhsT=v_sb[mt][:, hg * VW:(hg + 1) * VW],
                            rhs=et[:, nh * N_TOK + h * 512:
                                   nh * N_TOK + (h + 1) * 512],
                            start=(mt == 0), stop=(mt == NP_T - 1),
                        )
                    if stop:
                        if nh == 0:
                            inflight[("stg", p, h)] = stgp.tile(
                                [VW, N_TOK], F32, tag="stg", name=f"stg{hg}")
                        stg = inflight[("stg", p, h)]
                        if use_scalar_evict:
                            nc.scalar.copy(
                                out=stg[:, nh * 512:(nh + 1) * 512],
                                in_=pvt[0:VW, :])
                        else:
                            nc.vector.tensor_copy(
                                out=stg[:, nh * 512:(nh + 1) * 512],
                                in_=pvt[0:VW, :])
                        del inflight[key]
                        if nh == 1:
                            normalize_evict(p, h, stg)
                            del inflight[("stg", p, h)]
                            done = inflight.setdefault(("norm", p), set())
                            done.add(h)
                            if len(done) == 2:
                                for mt in range(NP_T):
                                    del et_tiles[(p, mt)]

                def pv_chunk(p, slot8):
                    """Baseline slotting: 4 PV matmuls for pair p at slot8;
                    (h, nh) = slot8//4, (slot8//2)%2; half = slot8%2."""
                    h, nh = slot8 // 4, (slot8 // 2) % 2
                    half = slot8 % 2
                    mts = list(range(4 * half, 4 * half + 4))
                    pv_group(p, h, nh, mts, start=(half == 0),
                             stop=(half == 1), use_scalar_evict=False)

                # filler: QKT for the next pair, nh-sequential so the psum
                # is a single [128, 512] bank. Slot s of 8:
                #   0: qt-nh0 k0-3   1: qt-nh0 k4-7 + evict
                #   2: qt-nh1 k0-3   3: qt-nh1 k4-7 + evict
                #   4: kt-nh0 k0-3   5: kt-nh0 k4-7 + evict
                #   6: kt-nh1 k0-3   7: kt-nh1 k4-7 + evict
                def filler_chunk(pnext, s, wq_next, wk_next):
                    is_kt = s >= 4
                    nh = (s // 2) % 2
                    khalf = s % 2
                    j = (DP + pnext) if is_kt else pnext
                    wblk = wk_next if is_kt else wq_next
                    pskey = ("fill", pnext)
                    tkey = ("fillt", pnext, is_kt)
                    if khalf == 0:
                        inflight[pskey] = pfill.tile(
                            [P, 512], F32, tag="pfill", name=f"psf{j}_{nh}")
                        if nh == 0:
                            inflight[tkey] = qktp.tile(
                                [P, N_TOK], _DT, tag="qkt", name=f"qkt{j}")
                    ps = inflight[pskey]
                    for k in range(khalf * 4, khalf * 4 + 4):
                        nc.tensor.matmul(
                            ps[:],
                            lhsT=wblk[:, k * P:(k + 1) * P],
                            rhs=xs(k)[:, nh * 512:(nh + 1) * 512],
                            start=(k == 0), stop=(k == DP - 1),
                        )
                    if khalf == 1:
                        t = inflight[tkey]
                        nc.vector.tensor_copy(
                            out=t[:, nh * 512:(nh + 1) * 512], in_=ps[:])
                        del inflight[pskey]
                        if nh == 1:
                            del inflight[tkey]
                            return t
                    return None

                def emit_st_exp(p, mt):
                    """S^T + exp for (p, mt): two [128, 1024] psum chunks
                    (nh-major), each exp'd separately so the next slot's
                    S^T only waits on the matching chunk's exp."""
                    et = etp.tile([P, 2 * N_TOK], _DT, tag="et",
                                  name=f"et{p}_{mt}")
                    for nh in range(2):
                        st = pstp.tile([P, N_TOK], F32, tag="pst",
                                       name=f"st{p}_{mt}_{nh}")
                        for h in range(2):
                            nc.tensor.matmul(
                                st[:, h * 512:(h + 1) * 512],
                                lhsT=kt_cur[h * D_K:(h + 1) * D_K,
                                            mt * P:(mt + 1) * P],
                                rhs=qt_cur[h * D_K:(h + 1) * D_K,
                                           nh * 512:(nh + 1) * 512],
                                start=True, stop=True,
                                tile_position=(h * D_K, 0),
                            )
                        nc.scalar.activation(
                            et[:, nh * N_TOK:(nh + 1) * N_TOK], st[:],
                            mybir.ActivationFunctionType.Exp,
                            scale=float(SCALE))
                    et_tiles[(p, mt)] = et

                # projection wave A: pairs 0-3 + bias -> oacc (bf16 SBUF),
                # emitted 2 js in loop 5, 2 in loop 6, 4 in loop 7. Runs
                # as two [128,512] nh-halves in the 1-bank pwa pool.
                oacc_all = wvoa.tile([P, DP * DIM], _DT, tag="wvoa",
                                     name="oacc")

                def oaccs(j):
                    return oacc_all[:, j * DIM:(j + 1) * DIM]

                def wave_a(j):
                    for nh in range(2):
                        ps = pwa.tile([P, 512], F32, tag="pwa",
                                      name=f"pswa{j}_{nh}")
                        for p in range(4):
                            nc.tensor.matmul(
                                ps[:],
                                lhsT=attnT_sb[p][:, j * P:(j + 1) * P],
                                rhs=wouts(p)[:, nh * 512:(nh + 1) * 512],
                                start=(p == 0), stop=(p == 3),
                            )
                        nc.vector.tensor_add(
                            out=oaccs(j)[:, nh * 512:(nh + 1) * 512],
                            in0=ps[:],
                            in1=bias_bc[:, nh * 512:(nh + 1) * 512])

                # ---- the pair loop ----
                for p in range(NPAIRS):
                    qt_next = kt_next = None
                    if p + 1 < NPAIRS:
                        wq_next = wqkb.tile([P, DIM], _DT, tag="wq",
                                            name=f"wq{p + 1}")
                        nc.sync.dma_start(wq_next[:], wq_block(p + 1))
                        wk_next = wqkb.tile([P, DIM], _DT, tag="wk",
                                            name=f"wk{p + 1}")
                        nc.sync.dma_start(wk_next[:], wq_block(DP + p + 1))
                    if p == 2:
                        issue_wout_bias()
                    for mt in range(NP_T):
                        if p + 1 < NPAIRS:
                            t = filler_chunk(p + 1, mt, wq_next, wk_next)
                            if t is not None:
                                if mt == 3:
                                    qt_next = t
                                else:
                                    kt_next = t
                        emit_st_exp(p, mt)
                        if p == 0:
                            emit_v_chunk(mt, 0)
                        if p > 0:
                            pv_chunk(p - 1, mt)
                        if 1 <= p <= 4 and mt in (5, 7):
                            # V nh1 chunks: j = 2*(p-1) + (mt==7)
                            emit_v_chunk(2 * (p - 1) + (mt == 7), 1)
                        if p in (5, 6) and mt in (6, 7):
                            wave_a(2 * (p - 5) + (mt == 7))
                        if p == NPAIRS - 1:
                            if mt % 2 == 1:
                                wave_a(4 + mt // 2)
                            if mt >= 4 and mt <= 6:
                                # pair-7 h0/nh0 PV: mts 0-5 inside the loop
                                m0 = 2 * (mt - 4)
                                pv_group(7, 0, 0, [m0, m0 + 1],
                                         start=(mt == 4), stop=False,
                                         use_scalar_evict=False)
                    qt_cur, kt_cur = qt_next, kt_next

                # ==== drain ====
                # finish pair-7 PV (stg evicts on the now-idle ScalarE),
                # wave B (pairs 4-6 -> oacc2) and wave C (pair 7 -> out)
                # pipelined behind it so the normalize chains and the out
                # DMAs hide under wave matmuls.
                with tc.tile_pool(name="ev", bufs=2) as ev:
                    oacc2 = {}

                    def wave_b(j):
                        ps = inflight[("wb", j)] = pstp.tile(
                            [P, N_TOK], F32, tag="pst", name=f"psb{j}")
                        for p in (4, 5, 6):
                            for nh in range(2):
                                nc.tensor.matmul(
                                    ps[:, nh * 512:(nh + 1) * 512],
                                    lhsT=attnT_sb[p][:, j * P:(j + 1) * P],
                                    rhs=wouts(p)[:, nh * 512:(nh + 1) * 512],
                                    start=(p == 4), stop=(p == 6),
                                )

                    def wave_b_evict(j):
                        ps = inflight.pop(("wb", j))
                        o2 = oacc2[j] = oap.tile([P, DIM], _DT, tag="oacc2",
                                                 name=f"oacc2_{j}")
                        nc.vector.tensor_add(out=o2[:], in0=ps[:],
                                             in1=oaccs(j))

                    def wave_c(j):
                        o2 = oacc2.pop(j)
                        o = ev.tile([P, DIM], _DT, tag="out", name=f"o{j}")
                        for nh in range(2):
                            ps = pgen.tile([P, 512], F32, tag="pv",
                                           name=f"psc{j}_{nh}")
                            nc.tensor.matmul(
                                ps[:],
                                lhsT=attnT_sb[7][:, j * P:(j + 1) * P],
                                rhs=wouts(7)[:, nh * 512:(nh + 1) * 512],
                                start=True, stop=True,
                            )
                            nc.vector.tensor_add(
                                out=o[:, nh * 512:(nh + 1) * 512],
                                in0=ps[:],
                                in1=o2[:, nh * 512:(nh + 1) * 512])
                        dma = nc.sync.dma_start if j % 2 else \
                            nc.gpsimd.dma_start
                        dma(out[j * P:(j + 1) * P, :], o[:])

                    # pair-7 h0/nh0: finish mts 6,7 and evict (ScalarE)
                    pv_group(7, 0, 0, [6, 7], start=False, stop=True,
                             use_scalar_evict=True)
                    # h0/nh1 full -> stg(7,h0) complete -> normalize h0
                    pv_group(7, 0, 1, [0, 1, 2, 3], start=True, stop=False,
                             use_scalar_evict=True)
                    pv_group(7, 0, 1, [4, 5, 6, 7], start=False, stop=True,
                             use_scalar_evict=True)
                    wave_b(0)
                    pv_group(7, 1, 0, [0, 1, 2, 3], start=True, stop=False,
                             use_scalar_evict=True)
                    wave_b_evict(0)
                    wave_b(1)
                    pv_group(7, 1, 0, [4, 5, 6, 7], start=False, stop=True,
                             use_scalar_evict=True)
                    wave_b_evict(1)
                    wave_b(2)
                    pv_group(7, 1, 1, [0, 1, 2, 3], start=True, stop=False,
                             use_scalar_evict=True)
                    wave_b_evict(2)
                    pv_group(7, 1, 1, [4, 5, 6, 7], start=False, stop=True,
                             use_scalar_evict=True)
                    for j in range(3, NP_T):
                        wave_b(j)
                        wave_b_evict(j)
                        if j >= 4:
                            wave_c(j - 4)
                    for j in range(NP_T - 4, NP_T):
                        wave_c(j)

    nc.compile()
    return nc


_NC_CACHE = None


def _get_program():
    global _NC_CACHE
    if _NC_CACHE is None:
        _NC_CACHE = build_program()
    return _NC_CACHE


def make_in_maps(x, w_qkv, w_out, b_out):
    w_qkv_c = np.ascontiguousarray(w_qkv).astype(_NPDT)
    w_out_c = np.ascontiguousarray(w_out).astype(_NPDT)
    b_out_c = np.ascontiguousarray(b_out).astype(np.float32)
    in_maps = []
    for b in range(N_CORES):
        xTb = np.ascontiguousarray(np.asarray(x[b]).T).astype(_NPDT)
        in_maps.append({
            "xT": xTb,
            "w_qkv": w_qkv_c,
            "w_out": w_out_c,
            "b_out": b_out_c,
        })
    return in_maps


def kernel(x, w_qkv, w_out, b_out):
    nc = _get_program()
    in_maps = make_in_maps(x, w_qkv, w_out, b_out)
    res = run_bass_kernel_spmd(nc, in_maps, list(range(N_CORES)))
    outs = [np.asarray(r["out"], dtype=np.float32) for r in res.results]
    return np.stack(outs, axis=0)


# revision 26
# speedup vs baseline: 1.2537x; 1.2537x over previous
"""ViT attention block (B=8, N=1024, dim=1024, heads=16, d_k=64) on 8 trn2 NeuronCores.

Sharding: data-parallel over batch (1 batch per core), weights replicated.
No collectives needed; each core computes its batch's full attention output.

Per-core algorithm (all matmuls on TensorE contract over the partition dim):
  - host pre-transposes x[b] -> xT [dim, tokens] so QKV projections can use
    w_qkv (natural layout) as the stationary operand.
  - QT/KT = (w_qkv[:, :2048]).T @ xT  -> [2048, tokens]; head pair 2t,2t+1
    lives in partition-tile t ([128, 1024]), i.e. heads' d_k=64 rows stacked.
  - V = xT.T @ w_qkv[:, 2048:]       -> [tokens, 1024], stored with a
    constant-1 column appended per head (65 cols/head) so the PV matmul
    produces softmax row-sums for free.
  - per head pair: S^T[m,n] = (KT tile).T @ QT (K=64 contraction; the two
    heads run as concurrent row-group matmuls via tile_position). exp via
    ScalarE streams per [128,1024] chunk into ET bf16. ScalarE exp is the
    per-pair pacing engine (16 x 1.15us = 18.4us/pair), so every pair loop
    is packed with >= that much TensorE work (V-nh1 chunks ride loops 1-4,
    out-projection wave A rides loops 5-7).
  - PV: out^T[d'+1, n] = V'.T @ E^T accumulated over m tiles; row 64 is the
    softmax denominator. Stage to SBUF, reciprocal on a [128, 8] reshape,
    DRAM-bounce broadcast back, normalize multiplies on GpSimd/DVE.
  - final = attnT.T @ w_out + b_out in three waves: wave A (pairs 0-3,
    bias folded) -> oacc during loops 5-7; wave B (pairs 4-6) -> oacc2 and
    wave C (pair 7) -> out in the drain, pipelined so the out DMAs overlap
    the wave MMs. Output rides DRAM as bf16 (halves the 4MB tail DMA).

Startup: ~28 garbage warm-up matmuls issue immediately after the framework
preamble so the PE HAM clock-gate reaches 2.4 GHz before the first real
matmul. Input DMAs are priority-ordered: xT (sync), wq0 (scalar, 2 desc),
wk0 (vector, 2 desc), wv nh0-half then nh1-half (gpsimd); wout/bias issue
at loop 2 so they don't steal startup HBM bandwidth.
"""

import os
import numpy as np
import ml_dtypes

import concourse.bass as bass
from concourse import bacc
import concourse.mybir as mybir
import concourse.tile as tile
from concourse.bass_utils import run_bass_kernel_spmd

P = 128
N_TOK = 1024
DIM = 1024
HEADS = 16
D_K = 64
N_CORES = 8
SCALE = D_K ** -0.5  # 0.125

NP_T = N_TOK // P   # 8 token tiles
DP = DIM // P       # 8 dim tiles
NPAIRS = HEADS // 2  # 8 head pairs
VW = D_K + 1        # 65: V columns per head incl. ones column

# matmul operand dtype: "bf16" | "fp32r" | "fp32"
MM_DTYPE = os.environ.get("KERNEL_MM_DTYPE", "bf16")
_DT = {
    "bf16": mybir.dt.bfloat16,
    "fp32r": mybir.dt.float32r,
    "fp32": mybir.dt.float32,
}[MM_DTYPE]
_NPDT = {"bf16": ml_dtypes.bfloat16, "fp32r": np.float32, "fp32": np.float32}[MM_DTYPE]

F32 = mybir.dt.float32

N_WARMUP = 28  # garbage matmuls to flip the HAM clock gate early


def build_program():
    nc = bacc.Bacc("TRN2", target_bir_lowering=False, debug=False)

    xT = nc.dram_tensor("xT", [DIM, N_TOK], _DT, kind="ExternalInput").ap()
    wqkv = nc.dram_tensor("w_qkv", [DIM, 3 * DIM], _DT, kind="ExternalInput").ap()
    wout = nc.dram_tensor("w_out", [DIM, DIM], _DT, kind="ExternalInput").ap()
    bout = nc.dram_tensor("b_out", [DIM], F32, kind="ExternalInput").ap()
    out = nc.dram_tensor("out", [N_TOK, DIM], _DT, kind="ExternalOutput").ap()
    # reciprocal'd softmax denominator bounce rows (one per head)
    rs_dram = nc.dram_tensor("rs_scratch", [HEADS, N_TOK], F32).ap()

    def wq_block(j):
        """One DMA-able view of w_qkv column block j (128 cols) across all
        1024 rows: [p=128, k=8, c=128] -> lands as an SBUF [128, 1024] tile
        whose k-th 128-col slice is w_qkv[k*128:(k+1)*128, j*128:(j+1)*128]."""
        return bass.AP(
            tensor=wqkv.tensor, offset=wqkv.offset + j * P,
            ap=[[3 * DIM, P], [P * 3 * DIM, DP], [1, P]],
        )

    def wq_halfblock(j, khalf):
        """Half of wq_block: k-tiles khalf*4..khalf*4+3 of column block j."""
        return bass.AP(
            tensor=wqkv.tensor,
            offset=wqkv.offset + j * P + khalf * 4 * P * 3 * DIM,
            ap=[[3 * DIM, P], [P * 3 * DIM, 4], [1, P]],
        )

    def wv_halfsrc(k0, nh):
        """V-weight nh-half for k-tiles k0, k0+1: [128, 2*512]."""
        return bass.AP(
            tensor=wqkv.tensor,
            offset=wqkv.offset + k0 * P * 3 * DIM + 2 * DIM + nh * 512,
            ap=[[3 * DIM, P], [P * 3 * DIM, 2], [1, 512]])

    with tile.TileContext(nc) as tc:
        with (
            tc.tile_pool(name="persist", bufs=1) as persist,
            tc.tile_pool(name="qkt", bufs=4) as qktp,
            tc.tile_pool(name="wqkb", bufs=4) as wqkb,
            tc.tile_pool(name="wvoa", bufs=1) as wvoa,
            tc.tile_pool(name="et", bufs=16) as etp,
            tc.tile_pool(name="stg", bufs=2) as stgp,
            tc.tile_pool(name="small", bufs=2) as small,
            tc.tile_pool(name="oap", bufs=4) as oap,
        ):
            v_sb = []      # per token-tile: [128, 16*65]
            attnT_sb = []  # per pair: [128, 1024] = two heads' [64, n]
            for j in range(NP_T):
                v_sb.append(persist.tile([P, HEADS * VW], _DT, tag=f"v{j}",
                                         name=f"v{j}"))
            for p in range(NPAIRS):
                attnT_sb.append(persist.tile([P, N_TOK], _DT, tag=f"attnT{p}",
                                             name=f"attnT{p}"))

            # ---- input DMAs, priority ordered across 4 queues ----
            # sync: xT (the phase-1 pacing stream); scalar: wq0 halves;
            # vector: wk0 halves; gpsimd: wv nh0-half then nh1-half.
            # wout/bias issue later (loop 2) so they don't steal startup BW.
            # xT is the phase-1 pacing stream; a single queue only gets a
            # fraction of the HBM bandwidth, so the 8 chunks interleave
            # across all three DMA queues with the small wq0/wk0 halves.
            xT_all = persist.tile([P, DP * N_TOK], _DT, tag="xT", name="xT")

            def xs(k):
                return xT_all[:, k * N_TOK:(k + 1) * N_TOK]

            def xdma(eng, k):
                eng.dma_start(xT_all[:, k * N_TOK:(k + 1) * N_TOK],
                              xT[k * P:(k + 1) * P, :])

            wq_cur = wqkb.tile([P, DIM], _DT, tag="wq", name="wq0")
            wk_cur = wqkb.tile([P, DIM], _DT, tag="wk", name="wk0")
            wv_all = wvoa.tile([P, DP * DIM], _DT, tag="wvoa", name="wv")

            xdma(nc.sync, 0)
            nc.scalar.dma_start(wq_cur[:, 0:512], wq_halfblock(0, 0))
            nc.gpsimd.dma_start(wk_cur[:, 0:512], wq_halfblock(DP, 0))
            xdma(nc.sync, 2)
            xdma(nc.scalar, 1)
            xdma(nc.gpsimd, 3)
            xdma(nc.sync, 4)
            nc.scalar.dma_start(wq_cur[:, 512:1024], wq_halfblock(0, 1))
            nc.gpsimd.dma_start(wk_cur[:, 512:1024], wq_halfblock(DP, 1))
            xdma(nc.sync, 6)
            xdma(nc.scalar, 5)
            xdma(nc.gpsimd, 7)

            # wv layout: [128, 2*4096]: nh-half major, then k-major 512-col
            # blocks: wv_all[:, nh*4096 + k*512 : +512] = w_qkv V columns
            # [k*128:(k+1)*128, 2048+nh*512 : 2048+(nh+1)*512]
            for nh in range(2):
                for k0 in range(0, DP, 2):
                    nc.gpsimd.dma_start(
                        wv_all[:, nh * 4096 + k0 * 512: nh * 4096 + (k0 + 2) * 512],
                        wv_halfsrc(k0, nh))

            def wvs(k, nh):
                return wv_all[:, nh * 4096 + k * 512:nh * 4096 + (k + 1) * 512]

            wout_all = persist.tile([P, DP * DIM], _DT, tag="wout",
                                    name="wout")
            bias_bc = persist.tile([P, DIM], F32, tag="bias")

            def issue_wout_bias():
                for k0 in range(0, DP, 2):
                    src = bass.AP(
                        tensor=wout.tensor, offset=wout.offset + k0 * P * DIM,
                        ap=[[DIM, P], [P * DIM, 2], [1, DIM]])
                    nc.gpsimd.dma_start(
                        wout_all[:, k0 * DIM:(k0 + 2) * DIM], src)
                bias_in = bass.AP(tensor=bout.tensor, offset=bout.offset,
                                  ap=[[0, P]] + list(bout.ap))
                nc.gpsimd.dma_start(bias_bc[:], bias_in)

            def wouts(k):
                return wout_all[:, k * DIM:(k + 1) * DIM]

            for j in range(NP_T):
                nc.vector.memset(
                    v_sb[j][:].rearrange("p (h x) -> p h x", x=VW)[:, :, D_K:],
                    1.0)

            # ============ phase 0: HAM warm-up. Garbage matmuls on a
            # memset tile into a scratch psum bank keep the PE busy from
            # the end of the framework preamble so the clock gate flips
            # to 8/8 before the first real matmul. ============
            with tc.tile_pool(name="pwarm", bufs=1, space="PSUM") as pwarm, \
                    tc.tile_pool(name="pq1", bufs=2, space="PSUM") as pq1:
                wtile = small.tile([P, P], _DT, tag="warm", name="warmsrc")
                nc.vector.memset(wtile[:], 0.0)
                wps = pwarm.tile([P, P], F32, tag="warm", name="warmps")
                for _ in range(N_WARMUP):
                    nc.tensor.matmul(wps[:], lhsT=wtile[:], rhs=wtile[:],
                                     start=True, stop=True)

                # ============ phase 1: pair-0 QT/KT, k-interleaved so both
                # psums accumulate as the xT / weight DMAs arrive ==========
                psq = pq1.tile([P, N_TOK], F32, tag="pq", name="psqk0")
                psk = pq1.tile([P, N_TOK], F32, tag="pq", name="psqk8")
                for k in range(DP):
                    for ps, wblk in ((psq, wq_cur), (psk, wk_cur)):
                        for nh in range(2):
                            nc.tensor.matmul(
                                ps[:, nh * 512:(nh + 1) * 512],
                                lhsT=wblk[:, k * P:(k + 1) * P],
                                rhs=xs(k)[:, nh * 512:(nh + 1) * 512],
                                start=(k == 0), stop=(k == DP - 1),
                            )
                    if k >= 2:
                        # filler dummies: absorb DMA-arrival jitter so the
                        # HAM activity window never sees the PE idle
                        for _ in range(2):
                            nc.tensor.matmul(wps[:], lhsT=wtile[:],
                                             rhs=wtile[:],
                                             start=True, stop=True)
                qt_cur = qktp.tile([P, N_TOK], _DT, tag="qkt", name="qkt0")
                nc.vector.tensor_copy(out=qt_cur[:], in_=psq[:])
                kt_cur = qktp.tile([P, N_TOK], _DT, tag="qkt", name="qkt8")
                nc.vector.tensor_copy(out=kt_cur[:], in_=psk[:])

            # ============ phase 2: pipelined attention ============
            with (
                tc.tile_pool(name="pst", bufs=2, space="PSUM") as pstp,
                tc.tile_pool(name="pfill", bufs=1, space="PSUM") as pfill,
                tc.tile_pool(name="pwa", bufs=1, space="PSUM") as pwa,
                tc.tile_pool(name="pgen", bufs=2, space="PSUM") as pgen,
            ):
                et_tiles = {}   # (pair, mt) -> ET tile [128, 2048]
                inflight = {}   # accumulation state for pv / filler psums

                def emit_v_chunk(j, nh):
                    """8 matmuls: V[j-tile, nh-half] = xT.T @ wv, evicted
                    into v_sb[j] (heads nh*8..nh*8+7, 64 cols each)."""
                    ps = pgen.tile([P, 512], F32, tag="pv",
                                   name=f"psv{j}_{nh}")
                    for k in range(DP):
                        nc.tensor.matmul(
                            ps[:],
                            lhsT=xs(k)[:, j * P:(j + 1) * P],
                            rhs=wvs(k, nh),
                            start=(k == 0), stop=(k == DP - 1),
                        )
                    nc.vector.tensor_copy(
                        out=v_sb[j][:].rearrange(
                            "p (h x) -> p h x", x=VW)[:, 8 * nh:8 * nh + 8,
                                                      :D_K],
                        in_=ps[:].rearrange("p (h d) -> p h d", d=D_K),
                    )

                def normalize_evict(p, h, stg):
                    """Normalize the staged PV result by the softmax
                    denominator (row 64) and write into attnT_sb[p].
                    Latency chains ride the gpsimd SWDGE queue; the last
                    pair uses the (by then idle) sync queue."""
                    hg = 2 * p + h
                    # pairs 6/7 normalize right before their consumers in
                    # the drain; the sync queue is idle then and has the
                    # lowest latency.
                    dma = nc.gpsimd.dma_start if p < NPAIRS - 2 else \
                        nc.sync.dma_start
                    rsp = small.tile([P, NP_T], F32, tag="rsp",
                                     name=f"rsp{hg}")
                    dma(rsp[:], stg[D_K:VW, :].rearrange(
                        "o (p i) -> o p i", p=P))
                    rsq = small.tile([P, NP_T], F32, tag="rsq",
                                     name=f"rsq{hg}")
                    nc.vector.reciprocal(rsq[:], rsp[:])
                    dma(rs_dram[hg].rearrange("(p i) -> p i", p=P), rsq[:])
                    rs_row = rs_dram[hg:hg + 1, :]
                    rs_bc = bass.AP(tensor=rs_row.tensor, offset=rs_row.offset,
                                    ap=[[0, D_K], list(rs_row.ap)[-1]])
                    rrec = small.tile([D_K, N_TOK], F32, tag="rrec",
                                      name=f"rrec{hg}")
                    dma(rrec[:], rs_bc)
                    # normalize multiplies run on GpSimd (SBUF-only
                    # operands) to keep the DVE free for psum evictions
                    if h == 0:
                        nc.gpsimd.tensor_mul(out=attnT_sb[p][0:D_K, :],
                                             in0=stg[0:D_K, :], in1=rrec[:])
                    else:
                        tmp = small.tile([D_K, N_TOK], _DT, tag="oddtmp",
                                         name=f"oddtmp{hg}")
                        nc.gpsimd.tensor_mul(out=tmp[:],
                                             in0=stg[0:D_K, :], in1=rrec[:])
                        dma(attnT_sb[p][D_K:P, :], tmp[:])

                def pv_group(p, h, nh, mts, start, stop, use_scalar_evict,
                             pool=None, pooltag="pv"):
                    """PV matmuls for (pair, head, nh) over the given mt
                    list, accumulated in a pgen psum. On stop: evict into
                    the stg half; on nh==1 stop: launch the normalize."""
                    hg = 2 * p + h
                    key = (p, h, nh)
                    if start:
                        inflight[key] = (pool or pgen).tile(
                            [P, 512], F32, tag=pooltag,
                            name=f"pv{p}_{h}_{nh}")
                    pvt = inflight[key]
                    for mt in mts:
                        et = et_tiles[(p, mt)]
                        nc.tensor.matmul(
                            pvt[0:VW, :],
                            lhsT=v_sb[mt][:, hg * VW:(hg + 1) * VW],
                            rhs=et[:, nh * N_TOK + h * 512:
                                   nh * N_TOK + (h + 1) * 512],
                            start=(mt == 0), stop=(mt == NP_T - 1),
                        )
                    if stop:
                        if nh == 0:
                            inflight[("stg", p, h)] = stgp.tile(
                                [VW, N_TOK], F32, tag="stg", name=f"stg{hg}")
                        stg = inflight[("stg", p, h)]
                        if use_scalar_evict:
                            nc.scalar.copy(
                                out=stg[:, nh * 512:(nh + 1) * 512],
                                in_=pvt[0:VW, :])
                        else:
                            nc.vector.tensor_copy(
                                out=stg[:, nh * 512:(nh + 1) * 512],
                                in_=pvt[0:VW, :])
                        del inflight[key]
                        if nh == 1:
                            normalize_evict(p, h, stg)
                            del inflight[("stg", p, h)]
                            done = inflight.setdefault(("norm", p), set())
                            done.add(h)
                            if len(done) == 2:
                                for mt in range(NP_T):
                                    del et_tiles[(p, mt)]

                def pv_chunk(p, slot8):
                    """4 PV matmuls for pair p at slot8. h=1 runs first:
                    its normalize writes attnT[64:] through an extra DMA
                    hop, so its chain gets a head start before the h=0
                    (direct DVE write) chain."""
                    h, nh = 1 - slot8 // 4, (slot8 // 2) % 2
                    half = slot8 % 2
                    mts = list(range(4 * half, 4 * half + 4))
                    pv_group(p, h, nh, mts, start=(half == 0),
                             stop=(half == 1), use_scalar_evict=False)

                # filler: QKT for the next pair, nh-sequential so the psum
                # is a single [128, 512] bank. Slot s of 8:
                #   0: qt-nh0 k0-3   1: qt-nh0 k4-7 + evict
                #   2: qt-nh1 k0-3   3: qt-nh1 k4-7 + evict
                #   4: kt-nh0 k0-3   5: kt-nh0 k4-7 + evict
                #   6: kt-nh1 k0-3   7: kt-nh1 k4-7 + evict
                def filler_chunk(pnext, s, wq_next, wk_next):
                    is_kt = s >= 4
                    nh = (s // 2) % 2
                    khalf = s % 2
                    j = (DP + pnext) if is_kt else pnext
                    wblk = wk_next if is_kt else wq_next
                    pskey = ("fill", pnext)
                    tkey = ("fillt", pnext, is_kt)
                    if khalf == 0:
                        inflight[pskey] = pfill.tile(
                            [P, 512], F32, tag="pfill", name=f"psf{j}_{nh}")
                        if nh == 0:
                            inflight[tkey] = qktp.tile(
                                [P, N_TOK], _DT, tag="qkt", name=f"qkt{j}")
                    ps = inflight[pskey]
                    for k in range(khalf * 4, khalf * 4 + 4):
                        nc.tensor.matmul(
                            ps[:],
                            lhsT=wblk[:, k * P:(k + 1) * P],
                            rhs=xs(k)[:, nh * 512:(nh + 1) * 512],
                            start=(k == 0), stop=(k == DP - 1),
                        )
                    if khalf == 1:
                        t = inflight[tkey]
                        nc.vector.tensor_copy(
                            out=t[:, nh * 512:(nh + 1) * 512], in_=ps[:])
                        del inflight[pskey]
                        if nh == 1:
                            del inflight[tkey]
                            return t
                    return None

                def emit_st_exp(p, mt):
                    """S^T + exp for (p, mt): two [128, 1024] psum chunks
                    (nh-major), each exp'd separately so the next slot's
                    S^T only waits on the matching chunk's exp."""
                    et = etp.tile([P, 2 * N_TOK], _DT, tag="et",
                                  name=f"et{p}_{mt}")
                    for nh in range(2):
                        st = pstp.tile([P, N_TOK], F32, tag="pst",
                                       name=f"st{p}_{mt}_{nh}")
                        for h in range(2):
                            nc.tensor.matmul(
                                st[:, h * 512:(h + 1) * 512],
                                lhsT=kt_cur[h * D_K:(h + 1) * D_K,
                                            mt * P:(mt + 1) * P],
                                rhs=qt_cur[h * D_K:(h + 1) * D_K,
                                           nh * 512:(nh + 1) * 512],
                                start=True, stop=True,
                                tile_position=(h * D_K, 0),
                            )
                        nc.scalar.activation(
                            et[:, nh * N_TOK:(nh + 1) * N_TOK], st[:],
                            mybir.ActivationFunctionType.Exp,
                            scale=float(SCALE))
                    et_tiles[(p, mt)] = et

                # projection wave A: pairs 0-3 + bias -> oacc (bf16 SBUF),
                # emitted 2 js in loop 5, 2 in loop 6, 4 in loop 7. Runs
                # as two [128,512] nh-halves in the 1-bank pwa pool.
                oacc_all = wvoa.tile([P, DP * DIM], _DT, tag="wvoa",
                                     name="oacc")

                def oaccs(j):
                    return oacc_all[:, j * DIM:(j + 1) * DIM]

                def wave_a(j, nh):
                    ps = pwa.tile([P, 512], F32, tag="pwa",
                                  name=f"pswa{j}_{nh}")
                    for p in range(4):
                        nc.tensor.matmul(
                            ps[:],
                            lhsT=attnT_sb[p][:, j * P:(j + 1) * P],
                            rhs=wouts(p)[:, nh * 512:(nh + 1) * 512],
                            start=(p == 0), stop=(p == 3),
                        )
                    nc.vector.tensor_add(
                        out=oaccs(j)[:, nh * 512:(nh + 1) * 512],
                        in0=ps[:],
                        in1=bias_bc[:, nh * 512:(nh + 1) * 512])

                # ---- the pair loop ----
                for p in range(NPAIRS):
                    qt_next = kt_next = None
                    if p + 1 < NPAIRS:
                        wq_next = wqkb.tile([P, DIM], _DT, tag="wq",
                                            name=f"wq{p + 1}")
                        nc.sync.dma_start(wq_next[:], wq_block(p + 1))
                        wk_next = wqkb.tile([P, DIM], _DT, tag="wk",
                                            name=f"wk{p + 1}")
                        nc.sync.dma_start(wk_next[:], wq_block(DP + p + 1))
                    if p == 2:
                        issue_wout_bias()
                    # slot order: PV first (inputs a pair old, always ready),
                    # filler next (weights prefetched a loop ahead), extras,
                    # then S^T last -- S^T waits on exp freeing its psum
                    # bank, and the PE queue is FIFO, so a stalled S^T at
                    # the head would block everything emitted after it.
                    for mt in range(NP_T):
                        if p > 0:
                            pv_chunk(p - 1, mt)
                        if p + 1 < NPAIRS:
                            t = filler_chunk(p + 1, mt, wq_next, wk_next)
                            if t is not None:
                                if mt == 3:
                                    qt_next = t
                                else:
                                    kt_next = t
                        if p == 0:
                            emit_v_chunk(mt, 0)
                        if 1 <= p <= 4 and mt in (5, 7):
                            # V nh1 chunks: j = 2*(p-1) + (mt==7)
                            emit_v_chunk(2 * (p - 1) + (mt == 7), 1)
                        if p in (5, 6) and mt >= 4:
                            # one [128,512] wave-A half per slot
                            wave_a(2 * (p - 5) + (mt - 4) // 2, mt % 2)
                        if p == NPAIRS - 1:
                            wave_a(4 + mt // 2, mt % 2)
                            if mt >= 4 and mt <= 6:
                                # pair-7 h1/nh0 PV: mts 0-5 inside the loop
                                m0 = 2 * (mt - 4)
                                pv_group(7, 1, 0, [m0, m0 + 1],
                                         start=(mt == 4), stop=False,
                                         use_scalar_evict=False)
                        emit_st_exp(p, mt)
                    qt_cur, kt_cur = qt_next, kt_next

                # ==== drain ====
                # finish pair-7 PV (stg evicts on the now-idle ScalarE),
                # wave B (pairs 4-6 -> oacc2) and wave C (pair 7 -> out)
                # pipelined behind it so the normalize chains and the out
                # DMAs hide under wave matmuls.
                with tc.tile_pool(name="ev", bufs=3) as ev:
                    oacc2 = {}

                    def wave_b(j, with_c=False):
                        """Pairs 4-6 (and pair 7 too when with_c) into one
                        [128,1024] psum group."""
                        ps = inflight[("wb", j)] = pstp.tile(
                            [P, N_TOK], F32, tag="pst", name=f"psb{j}")
                        pairs = (4, 5, 6, 7) if with_c else (4, 5, 6)
                        for p in pairs:
                            for nh in range(2):
                                nc.tensor.matmul(
                                    ps[:, nh * 512:(nh + 1) * 512],
                                    lhsT=attnT_sb[p][:, j * P:(j + 1) * P],
                                    rhs=wouts(p)[:, nh * 512:(nh + 1) * 512],
                                    start=(p == 4), stop=(p == pairs[-1]),
                                )

                    def wave_b_evict(j, final=False):
                        """final: psum holds pairs 4-7 -> o = ps + oacc,
                        DMA out. else: oacc2 = ps + oacc, pair 7 added
                        later by wave_c."""
                        ps = inflight.pop(("wb", j))
                        if final:
                            o = ev.tile([P, DIM], _DT, tag="out",
                                        name=f"o{j}")
                            nc.vector.tensor_add(out=o[:], in0=ps[:],
                                                 in1=oaccs(j))
                            nc.sync.dma_start(out[j * P:(j + 1) * P, :],
                                              o[:])
                        else:
                            o2 = oacc2[j] = oap.tile(
                                [P, DIM], _DT, tag="oacc2", name=f"oacc2_{j}")
                            nc.vector.tensor_add(out=o2[:], in0=ps[:],
                                                 in1=oaccs(j))

                    def wave_c(j):
                        o2 = oacc2.pop(j)
                        o = ev.tile([P, DIM], _DT, tag="out", name=f"o{j}")
                        for nh in range(2):
                            ps = pgen.tile([P, 512], F32, tag="pv",
                                           name=f"psc{j}_{nh}")
                            nc.tensor.matmul(
                                ps[:],
                                lhsT=attnT_sb[7][:, j * P:(j + 1) * P],
                                rhs=wouts(7)[:, nh * 512:(nh + 1) * 512],
                                start=True, stop=True,
                            )
                            nc.vector.tensor_add(
                                out=o[:, nh * 512:(nh + 1) * 512],
                                in0=ps[:],
                                in1=o2[:, nh * 512:(nh + 1) * 512])
                        nc.sync.dma_start(out[j * P:(j + 1) * P, :], o[:])

                    # pair-7 PV groups ordered so the drain's first matmuls
                    # only touch ET mts whose exp is already done (exp of
                    # mts 6,7 lands ~2us after the loop); both normalize
                    # chains then fly while wave B fills the PE. h0/nh0
                    # borrows the idle wave-A psum bank so three mts-0-3
                    # groups can be in flight before any [6,7] completes.
                    # nh0-stop evicts ride the (exp-backlogged) ScalarE;
                    # the norm-gating nh1-stop evicts ride the DVE.
                    pv_group(7, 1, 1, [0, 1, 2, 3], start=True, stop=False,
                             use_scalar_evict=True)
                    pv_group(7, 0, 0, [0, 1, 2, 3], start=True, stop=False,
                             use_scalar_evict=True, pool=pwa, pooltag="pwa")
                    pv_group(7, 1, 0, [6, 7], start=False, stop=True,
                             use_scalar_evict=True)
                    pv_group(7, 0, 1, [0, 1, 2, 3], start=True, stop=False,
                             use_scalar_evict=True)
                    pv_group(7, 1, 1, [4, 5, 6, 7], start=False, stop=True,
                             use_scalar_evict=False)
                    pv_group(7, 0, 0, [4, 5, 6, 7], start=False, stop=True,
                             use_scalar_evict=True, pool=pwa, pooltag="pwa")
                    pv_group(7, 0, 1, [4, 5, 6, 7], start=False, stop=True,
                             use_scalar_evict=False)
                    wave_b(0)
                    wave_b_evict(0)
                    wave_b(1)
                    wave_b_evict(1)
                    wave_b(2)
                    wave_b_evict(2)
                    # js 3-7: attnT_7 ready by now -> single combined
                    # psum group (pairs 4-7) per j, one evict, out DMA.
                    # js 0-2 get their pair-7 contribution via wave_c.
                    wave_b(3, with_c=True)
                    wave_c(0)
                    wave_b_evict(3, final=True)
                    wave_b(4, with_c=True)
                    wave_c(1)
                    wave_b_evict(4, final=True)
                    wave_b(5, with_c=True)
                    wave_c(2)
                    wave_b_evict(5, final=True)
                    wave_b(6, with_c=True)
                    wave_b_evict(6, final=True)
                    wave_b(7, with_c=True)
                    wave_b_evict(7, final=True)

    nc.compile()
    return nc


_NC_CACHE = None


def _get_program():
    global _NC_CACHE
    if _NC_CACHE is None:
        _NC_CACHE = build_program()
    return _NC_CACHE


def make_in_maps(x, w_qkv, w_out, b_out):
    w_qkv_c = np.ascontiguousarray(w_qkv).astype(_NPDT)
    w_out_c = np.ascontiguousarray(w_out).astype(_NPDT)
    b_out_c = np.ascontiguousarray(b_out).astype(np.float32)
    in_maps = []
    for b in range(N_CORES):
        xTb = np.ascontiguousarray(np.asarray(x[b]).T).astype(_NPDT)
        in_maps.append({
            "xT": xTb,
            "w_qkv": w_qkv_c,
            "w_out": w_out_c,
            "b_out": b_out_c,
        })
    return in_maps


def kernel(x, w_qkv, w_out, b_out):
    nc = _get_program()
    in_maps = make_in_maps(x, w_qkv, w_out, b_out)
    res = run_bass_kernel_spmd(nc, in_maps, list(range(N_CORES)))
    outs = [np.asarray(r["out"], dtype=np.float32) for r in res.results]
    return np.stack(outs, axis=0)
